# revision 17
# baseline (speedup 1.0000x reference)
"""Trainium2 Bass kernel for nn_BezierGlyph (retrieval_knn).

Math (matching the jax reference):
  pts  = cubic-bezier samples of clip(control_points, 0, 1)   # [512, 2]
  d_ij = |pixel_i - pts_j|
  m_i  = -logsumexp(-256 * d_i:) / 256                        # softmin
  out  = 1 - sigmoid((0.04 - m) * 200)                        # (1, 512, 512)

Strategy (sharding_hint: shard pixels, replicate points):
  * 512x512 pixels in 16x16 blocks (1024). Blocks with no sample point
    within min(0.151, dc_min + 0.070) of their bbox output exactly 1.0f
    and are skipped entirely (host writes the 1.0s). The ~700 live
    blocks are dealt round-robin (sorted by candidate count) over the
    8 cores; the shared SPMD schedule is the slot-wise max.
  * Coordinates are re-centered per block: d^2 = |q-c|^2 + |p-c|^2
    - 2(p-c).(q-c) via an 11-row bf16 limb contraction (2-limb splits
    suffice at these magnitudes; worst-case |noise| ~3e-7, clamped by
    the sqrt bias below). Up to 4 results (128-pixel subtiles) share
    one matmul: their 11-row groups stack on the contraction axis and
    the moving operand is block-diagonal — per-matmul overhead (~60ns)
    dominates at these sizes, so fewer/fatter matmuls win.
  * Two activation passes instead of three:
        v = sqrt(d^2 + 6e-7)        # Sqrt table
        w = exp(-256 * v)           # natural_log_exp table
    The kernel is split into a sqrt phase and an exp phase separated by
    a scheduler-only fence (tc.no_sync_barrier) so the final ACT stream
    needs exactly two table loads (a post-compile pass dedups the
    per-instruction reloads the stock pass inserts).
  * The Sqrt pass reads PSUM directly (no DVE clamp/copy pass); the Exp
    pass runs in place on SBUF; DVE only does the per-result row sums.
  * Inputs arrive as ONE dram tensor in three graduated chunks (first
    two batches, next three, rest) so the first matmul starts early and
    the bulk streams behind it.
  * Per 64-slot group: t = 8 + 0.78125*ln(sum + 1e-37);
    out = 1/(1 + exp(t)), DMA'd untransposed as [128, 2*nslots]
    (host transposes).
"""

import math

import ml_dtypes
import numpy as np

import concourse.bass as bass
import concourse.tile as tile
from concourse import bacc, mybir
from concourse.bass_utils import run_bass_kernel_spmd
from concourse.hw_specs import get_activation_tables

SIZE = 512
N_SAMPLES = 32
N_STROKES = 16
NPTS = N_STROKES * N_SAMPLES  # 512
SHARP = float(N_SAMPLES) * 8.0  # 256
STROKE_WIDTH = 0.04
OUT_SCALE = 8.0 / STROKE_WIDTH  # 200

NCORES = 8
BLK = 16  # block side in pixels
NB = SIZE // BLK  # 32
NBLOCKS = NB * NB  # 1024
PXB = BLK * BLK  # 256 pixels per block
SUBT = PXB // 128  # 2 subtiles of 128 pixels
HALFDIAG = BLK / SIZE * math.sqrt(2.0) / 2.0  # 0.0221
DELTA = 0.048  # points beyond dmin+DELTA are invisible (<=1e-3 out err)
CUTOFF = 0.103 + DELTA  # 0.103 = boring-pixel min_dist bound
PADG = 4  # candidate count granularity
KROWS = 11  # bf16 limb-product rows per result in the contraction
MMPACK = 4  # max results stacked per matmul (44 contraction rows)
GRP = 32  # slots per output group
SQ_BIAS = 6e-7  # clamps fp noise in d^2 (|noise| <~ 3e-7)

f32 = mybir.dt.float32
bf16 = mybir.dt.bfloat16
np_bf16 = ml_dtypes.bfloat16
AF = mybir.ActivationFunctionType

_prog_cache: dict = {}
_last_in_maps = None


def _bezier_points(control_points: np.ndarray) -> np.ndarray:
    """[16,4,2] control points -> [512,2] float64 curve samples."""
    pts = np.clip(control_points.astype(np.float64), 0.0, 1.0)
    t = np.linspace(0.0, 1.0, N_SAMPLES)[None, :, None]
    mt = 1.0 - t
    p0, p1, p2, p3 = (pts[:, k : k + 1, :] for k in range(4))
    cur = mt**3 * p0 + 3 * mt**2 * t * p1 + 3 * mt * t**2 * p2 + t**3 * p3
    return cur.reshape(-1, 2)


def _split2(x: np.ndarray):
    """2-way bf16 limb split (f64 in, 2x bf16 out; remainder ~2^-16 rel)."""
    a = x.astype(np_bf16)
    b = (x - a.astype(np.float64)).astype(np_bf16)
    return a, b


def _split3(x: np.ndarray):
    a = x.astype(np_bf16)
    r = x - a.astype(np.float64)
    b = r.astype(np_bf16)
    c = (r - b.astype(np.float64)).astype(np_bf16)
    return a, b, c


def _plan(k_sched: tuple[int, ...]):
    """Shared host/builder plan for a fixed per-slot candidate schedule.

    Batches: uniform-pitch psum tiles — a batch's 2*nslots results pack
    into one 4-bank tile at pitch P (bank r//rpb, offset (r%rpb)*P,
    rpb = 512//P); batches stay within one output group and stop
    extending when the pitch-lift would exceed 4/3x.

    Matmul groups: up to MMPACK consecutive same-bank results stack
    into one matmul (11 contraction rows each, block-diagonal moving
    operand).

    Input columns: one dram tensor, ordered chunk-by-chunk with each
    chunk's stationary (pix) columns first, then its moving (mov)
    columns, so each chunk is one contiguous DMA.
    """
    nslots = len(k_sched)
    batches = []
    pos = 0
    while pos < nslots:
        P = k_sched[pos]
        rpb = 512 // P
        lim = min((4 * rpb) // SUBT, nslots - pos, GRP - pos % GRP)
        j = 1
        while j < lim and 4 * k_sched[pos + j] >= 3 * P:
            j += 1
        batches.append((pos, j, P))
        pos += j

    lifted = list(k_sched)
    for start, nb, P in batches:
        for j in range(nb):
            lifted[start + j] = P

    # chunk id per batch: 0 = first two, 1 = next three, 2 = rest
    def chunk_of(bi):
        return 0 if bi < 2 else (1 if bi < 5 else 2)

    groups = []  # (bi, bank, r0, F, P)
    for bi, (start, nb, P) in enumerate(batches):
        rpb = 512 // P
        nr = SUBT * nb
        F = min(rpb, MMPACK)
        for b0 in range(0, nr, rpb):
            bend = min(b0 + rpb, nr)
            for r0 in range(b0, bend, F):
                groups.append((bi, b0 // rpb, r0, min(F, bend - r0), P))

    # vt column offset per batch (packed, no gaps)
    voff = []
    tot = 0
    for start, nb, P in batches:
        voff.append(tot)
        tot += SUBT * nb * P

    # Input column layout: DMA bandwidth scales with partition rows, so
    # the logical [44, *] operand blocks are packed into two partition
    # halves (rows 0-43 and 44-87) of an [88, totl] tensor, each chunk
    # split roughly evenly so one DMA covers both halves at full width.
    # (HW pattern rule: APs starting at a non-zero partition may touch
    # at most 32 partitions, so the operands all live at base 0; DMA
    # width is recovered by splitting each chunk across two DGE rings.)
    pix_loc = [None] * len(groups)  # (partition_offset, column)
    mov_loc = [None] * len(groups)
    chunk_slices = []
    cbase = 0
    for c in range(3):
        gs = [gi for gi, g in enumerate(groups) if chunk_of(g[0]) == c]
        start = cbase
        for gi in gs:
            pix_loc[gi] = (0, cbase)
            mov_loc[gi] = (0, cbase + 128)
            cbase += 128 + groups[gi][3] * groups[gi][4]
        if cbase > start:
            chunk_slices.append((start, cbase))
    totl = cbase

    return {
        "nslots": nslots,
        "batches": batches,
        "lifted": lifted,
        "groups": groups,
        "voff": voff,
        "vtot": tot,
        "pix_loc": pix_loc,
        "mov_loc": mov_loc,
        "chunk_slices": chunk_slices,
        "totl": totl,
        "ngroups": (nslots + GRP - 1) // GRP,
    }


def _build_program(k_sched: tuple[int, ...]):
    plan = _plan(k_sched)
    nslots = plan["nslots"]
    batches = plan["batches"]
    groups = plan["groups"]
    voff = plan["voff"]
    pix_loc = plan["pix_loc"]
    mov_loc = plan["mov_loc"]
    totl = plan["totl"]
    PROWS = KROWS * MMPACK  # 44

    nc = bacc.Bacc(None, target_bir_lowering=False, num_swdge_queues=4)

    inp_d = nc.dram_tensor("inp", [PROWS, totl], bf16, kind="ExternalInput")
    out_d = nc.dram_tensor("out", [128, nslots * SUBT], f32, kind="ExternalOutput")

    with tile.TileContext(nc) as tc:
        with (
            tc.tile_pool(name="io", bufs=1) as io,
            tc.tile_pool(name="vtp", bufs=1) as vtp,
            tc.tile_pool(name="acc", bufs=2) as acc,
            tc.tile_pool(name="fin", bufs=2) as fin,
            tc.tile_pool(name="psum", bufs=2, space="PSUM") as psum,
        ):
            inp_all = io.tile([PROWS, totl], bf16)
            # DMA rate scales with partition rows (44 here), so chunks
            # go out on parallel DGE rings: SP and ACT HWDGE for the
            # latency-critical first two, SP+Pool split for the bulk.
            # (SWDGE descriptor generation is slow — keep gpsimd's share
            # small.)
            cs = plan["chunk_slices"]
            rings = [nc.sync, nc.scalar, None]
            for ci, (c0, c1) in enumerate(cs):
                if ci < 2 and len(cs) > 2:
                    rings[ci].dma_start(inp_all[:, c0:c1], inp_d[:, c0:c1])
                else:
                    mid = (c0 + 2 * c1) // 3
                    nc.sync.dma_start(inp_all[:, c0:mid], inp_d[:, c0:mid])
                    nc.gpsimd.dma_start(inp_all[:, mid:c1], inp_d[:, mid:c1])
            b_sqb = io.tile([128, 1], f32)
            nc.vector.memset(b_sqb, SQ_BIAS)
            b_tiny = io.tile([128, 1], f32)
            nc.vector.memset(b_tiny, 1e-37)
            b_eight = io.tile([128, 1], f32)
            nc.vector.memset(b_eight, STROKE_WIDTH * OUT_SCALE)

            vt = vtp.tile([128, plan["vtot"]], f32)
            wt = vtp.tile([128, plan["vtot"]], f32)

            # ---- phase A: matmuls + sqrt (Sqrt table) ----
            gi = 0
            for bi, (start, nb, P) in enumerate(batches):
                rpb = 512 // P
                nr = SUBT * nb
                pt = psum.tile([128, 4, 512], f32, tag="ps")
                while gi < len(groups) and groups[gi][0] == bi:
                    _, bank, r0, F, _ = groups[gi]
                    o = (r0 % rpb) * P
                    pp, pc = pix_loc[gi]
                    mp, mc = mov_loc[gi]
                    nc.tensor.matmul(
                        pt[:, bank, o : o + F * P],
                        inp_all[pp : pp + KROWS * F, pc : pc + 128],
                        inp_all[mp : mp + KROWS * F, mc : mc + F * P],
                        start=True,
                        stop=True,
                    )
                    gi += 1
                nbf, rem = nr // rpb, nr % rpb
                vo = voff[bi]
                if nbf:
                    nc.scalar.activation(
                        vt[:, vo : vo + nbf * rpb * P].rearrange(
                            "p (b c) -> p b c", c=rpb * P
                        ),
                        pt[:, :nbf, : rpb * P],
                        AF.Sqrt,
                        bias=b_sqb[:],
                    )
                if rem:
                    nc.scalar.activation(
                        vt[:, vo + nbf * rpb * P : vo + nr * P],
                        pt[:, nbf, : rem * P],
                        AF.Sqrt,
                        bias=b_sqb[:],
                    )

            # ACT stream fence: all Sqrts schedule before any Exp/Ln so
            # exactly two table loads survive. Scheduler-only, no sems.
            tc.no_sync_barrier()

            # ---- phase B: exp + row sums + per-group finalization ----
            sums = None
            for bi, (start, nb, P) in enumerate(batches):
                g = start // GRP
                if start % GRP == 0:
                    sums = acc.tile([128, GRP * SUBT], f32, tag="sums")
                nr = SUBT * nb
                vo = voff[bi]
                nc.scalar.activation(
                    wt[:, vo : vo + nr * P],
                    vt[:, vo : vo + nr * P],
                    AF.Exp,
                    scale=-SHARP,
                )
                co = (start % GRP) * SUBT
                nc.vector.reduce_sum(
                    sums[:, co : co + nr],
                    wt[:, vo : vo + nr * P].rearrange("p (r k) -> p r k", k=P),
                    axis=mybir.AxisListType.X,
                )
                last_of_group = (
                    bi + 1 == len(batches) or batches[bi + 1][0] // GRP != g
                )
                if last_of_group:
                    n = min(GRP, nslots - g * GRP) * SUBT
                    zt = fin.tile([128, GRP * SUBT], f32, tag="z")
                    nc.scalar.activation(
                        zt[:, :n], sums[:, :n], AF.Ln, bias=b_tiny[:]
                    )
                    nc.scalar.activation(
                        zt[:, :n], zt[:, :n], AF.Exp, bias=b_eight[:],
                        scale=OUT_SCALE / SHARP,
                    )
                    nc.vector.tensor_scalar_add(zt[:, :n], zt[:, :n], 1.0)
                    # arg in [1, ~4e5]: no 0/denorm/inf edge cases
                    nc.vector.reciprocal_approx_fast(zt[:, :n], zt[:, :n])
                    c = g * GRP * SUBT
                    nc.sync.dma_start(out_d[:, c : c + n], zt[:, :n])

    nc.compile()

    # Keep one table load per phase: retarget the first load before a
    # Sqrt to the sqrt set, the first before an Exp/Ln to the combined
    # natural-log/exp set, and drop the redundant reloads in between.
    tables = list(get_activation_tables(nc.m.arch).items())
    sqrt_id = next(i for i, (_, fs) in enumerate(tables) if AF.Sqrt in fs)
    nl_id = next(i for i, (_, fs) in enumerate(tables) if {AF.Ln, AF.Exp} <= fs)
    for blk in nc.m.functions[0].blocks:
        cur = None
        pending = []
        for ins in list(blk.instructions):
            if isinstance(ins, mybir.InstLoadActFuncSet):
                pending.append(ins)
            elif isinstance(ins, mybir.InstActivation):
                need = sqrt_id if ins.func == AF.Sqrt else nl_id
                if pending:
                    if need != cur:
                        pending[0].act_func_set_id = need
                        for l in pending[1:]:
                            blk.instructions.remove(l)
                        cur = need
                    else:
                        for l in pending:
                            blk.instructions.remove(l)
                    pending = []
                else:
                    assert cur == need, "activation without table load"
        for l in pending:
            blk.instructions.remove(l)

    return nc, plan


def kernel(control_points: np.ndarray, pixel_grid: np.ndarray) -> np.ndarray:
    control_points = np.asarray(control_points, dtype=np.float32)
    pixel_grid = np.asarray(pixel_grid, dtype=np.float32)

    pts64 = _bezier_points(control_points)
    q64 = pts64.astype(np.float32).astype(np.float64)  # the fp32 values, exactly

    # ---- block geometry from the actual pixel grid ----
    pg = pixel_grid.reshape(SIZE, SIZE, 2)
    pblk = pg.reshape(NB, BLK, NB, BLK, 2).transpose(0, 2, 1, 3, 4)
    pblk = np.ascontiguousarray(pblk).reshape(NBLOCKS, PXB, 2).astype(np.float64)
    bxmin = pblk[:, :, 0].min(1)
    bxmax = pblk[:, :, 0].max(1)
    bymin = pblk[:, :, 1].min(1)
    bymax = pblk[:, :, 1].max(1)
    ccx = 0.5 * (bxmin + bxmax)
    ccy = 0.5 * (bymin + bymax)

    dx = np.maximum(np.maximum(bxmin[:, None] - q64[None, :, 0],
                               q64[None, :, 0] - bxmax[:, None]), 0.0)
    dy = np.maximum(np.maximum(bymin[:, None] - q64[None, :, 1],
                               q64[None, :, 1] - bymax[:, None]), 0.0)
    dc_min = np.sqrt((ccx[:, None] - q64[None, :, 0]) ** 2
                     + (ccy[:, None] - q64[None, :, 1]) ** 2).min(1)
    r_b = np.minimum(CUTOFF, dc_min + HALFDIAG + DELTA)
    cand = dx * dx + dy * dy < (r_b[:, None] + 5e-4) ** 2  # [NBLOCKS, 512]
    kcnt = cand.sum(1)
    nonempty = np.flatnonzero(kcnt > 0)
    kpad = (((kcnt + PADG - 1) // PADG) * PADG).astype(int)

    img = np.ones(SIZE * SIZE, dtype=np.float32)
    if len(nonempty) == 0:
        return img.reshape(1, SIZE, SIZE)

    # ---- deal blocks (sorted desc by padded count) round-robin ----
    order = nonempty[np.argsort(-kpad[nonempty], kind="stable")]
    nslots = (len(order) + NCORES - 1) // NCORES
    core_blocks = [order[c::NCORES] for c in range(NCORES)]
    k_sched = tuple(int(kpad[order[NCORES * i]]) for i in range(nslots))

    if k_sched not in _prog_cache:
        _prog_cache.clear()
        _prog_cache[k_sched] = _build_program(k_sched)
    nc, plan = _prog_cache[k_sched]

    # ---- shared per-block pix rows (block-centered, bf16 limbs) ----
    # rows: [1,1,1, -2x1,-2x1,-2x2, -2y1,-2y1,-2y2, pn1,pn2] pairing mov
    #       [qn1,qn2,qn3, qx1,qx2,qx1, qy1,qy2,qy1, 1,1]
    c_ne = np.stack([ccx[order], ccy[order]], axis=1)  # [n_live, 2]
    prel = pblk[order] - c_ne[:, None, :]  # [n_live, 256, 2]
    x1, x2 = _split2(prel[:, :, 0])
    y1, y2 = _split2(prel[:, :, 1])
    pn1, pn2 = _split2(prel[:, :, 0] ** 2 + prel[:, :, 1] ** 2)
    ones = np.ones_like(x1)
    pix_ne = np.stack(
        [ones, ones, ones,
         -2.0 * x1, -2.0 * x1, -2.0 * x2,
         -2.0 * y1, -2.0 * y1, -2.0 * y2,
         pn1, pn2], axis=1,
    ).astype(np_bf16)  # [n_live, 11, 256]

    # ---- per-core input arrays (block-diagonal matmul layout) ----
    groups = plan["groups"]
    batches = plan["batches"]
    pix_loc = plan["pix_loc"]
    mov_loc = plan["mov_loc"]
    in_maps = []
    for c in range(NCORES):
        blks = core_blocks[c]
        inp = np.zeros((KROWS * MMPACK, plan["totl"]), dtype=np_bf16)
        movs = {}  # slot -> [11, P] block (shared by both subtiles)
        for i in range(len(blks)):
            gi_b = NCORES * i + c
            b = blks[i]
            P = plan["lifted"][i]
            idx = np.flatnonzero(cand[b])
            qrel = q64[idx] - c_ne[gi_b][None, :]
            qx1, qx2 = _split2(qrel[:, 0])
            qy1, qy2 = _split2(qrel[:, 1])
            qn1, qn2, qn3 = _split3(qrel[:, 0] ** 2 + qrel[:, 1] ** 2)
            o1 = np.ones_like(qx1)
            m = np.zeros((KROWS, P), dtype=np_bf16)
            m[:, : len(idx)] = np.stack(
                [qn1, qn2, qn3, qx1, qx2, qx1, qy1, qy2, qy1, o1, o1])
            # far dummy candidate (qrel=(4,4): d>=5.6, exp underflows to 0)
            if P > len(idx):
                dn1, dn2, dn3 = _split3(np.array([32.0]))
                d1, d2 = _split2(np.array([4.0]))
                m[:, len(idx):] = np.array(
                    [dn1[0], dn2[0], dn3[0], d1[0], d2[0], d1[0],
                     d1[0], d2[0], d1[0], 1.0, 1.0], dtype=np_bf16)[:, None]
            movs[i] = m
        for g, (bi, bank, r0, F, P) in enumerate(groups):
            start = batches[bi][0]
            pp, pc = pix_loc[g]
            mp, mc = mov_loc[g]
            for j in range(F):
                r = r0 + j
                slot = start + r // SUBT
                st = r % SUBT
                i = slot  # per-core slot index
                if i >= len(blks):
                    continue  # dummy slot: zeros are safe (d^2 = 0)
                gi_b = NCORES * i + c
                inp[pp + KROWS * j : pp + KROWS * (j + 1), pc : pc + 128] = (
                    pix_ne[gi_b][:, st * 128 : (st + 1) * 128])
                inp[mp + KROWS * j : mp + KROWS * (j + 1),
                    mc + j * P : mc + (j + 1) * P] = movs[i]
        in_maps.append({"inp": inp})

    global _last_in_maps
    _last_in_maps = in_maps
    res = run_bass_kernel_spmd(nc, in_maps, core_ids=list(range(NCORES)))

    # ---- unshard: scatter block results back into the image ----
    by, bx = np.meshgrid(np.arange(NB), np.arange(NB), indexing="ij")
    lr, lc = np.meshgrid(np.arange(BLK), np.arange(BLK), indexing="ij")
    flat = ((by.reshape(-1, 1) * BLK + lr.reshape(-1)[None, :]) * SIZE
            + bx.reshape(-1, 1) * BLK + lc.reshape(-1)[None, :])  # [NBLOCKS, PXB]
    for c in range(NCORES):
        blks = core_blocks[c]
        o = res.results[c]["out"].T.reshape(nslots, PXB)  # [nslots, 256]
        img[flat[blks]] = o[: len(blks)]
    return img.reshape(1, SIZE, SIZE)
